# revision 12
# baseline (speedup 1.0000x reference)
"""Trainium2 Bass kernel for ModalityAwareDualAttention (dense_cnn).

Sharding: pure data-parallel over batch (32 -> 4 per core x 8 cores).
Per core: loop over P=3 parts; each part processes all BL=4 local batches.

Algebraic restructurings (exact up to fp assoc.):
  - depthwise scale/bias + 2x2-avg-pool 0.25 factor folded into Wq/Wk + biases
  - v computed transposed (vT = xd^T @ Wv^T) so no on-chip weight transpose
  - v-bias commutes through softmax (rows sum to 1); folded into upsample bias
    and (for the SE input) into fc1's bias
  - attention apply + bilinear 2x upsample + pa_gamma fused into two matmuls:
    up_s = vT.T @ (attn_n @ KT), KT = gamma * K_bilinear^T plus an extra
    column of column-means that yields mean(up) for the SE global-avg-pool
  - the SE gate is computed BEFORE the big upsample matmul (its input mean
    comes from the extra KT column via tiny per-block matmuls), so the
    upsample PSUM->SBUF copy fuses the whole output blend scaling:
      final = xp*(1 + mwc*cw) + up_s*(mw + mwc*cw),  mwc = mw*ca_gamma
  - weights quantized to fp8 e4m3 with per-(tensor,part) power-of-2 scales;
    descaling folded into biases, KT, and activation input scales
IO in bf16 (x converted on host, out converted back); residual path bf16.
"""

import numpy as np
import ml_dtypes

import concourse.bass as bass
import concourse.tile as tile
import concourse.mybir as mybir

F32 = mybir.dt.float32
BF16 = mybir.dt.bfloat16
FP8 = mybir.dt.float8e4
AF = mybir.ActivationFunctionType
ALU = mybir.AluOpType
DR = mybir.MatmulPerfMode.DoubleRow

N_CORES = 8
B, C, H, W, P = 32, 2048, 48, 24, 3
BL = B // N_CORES          # 4 local batches per core
IC = 128                   # q/k inter channels
C4 = 512                   # SE bottleneck
PH = H // P                # 16
HD, WD = PH // 2, W // 2   # 8, 12
N = HD * WD                # 96 attention tokens
HWP = PH * W               # 384 spatial positions per part
KC = C // 128              # 16 channel tiles
KC2 = KC // 2              # 8 channel-pair tiles (fp8 DoubleRow)

# const-pack column offsets (per part, [128, NCONST] f32)
O_QB = 0          # 1 col: Sq*qb   (IC=128 rows)
O_KB = 1          # 1 col: Sk*kb
O_ES = 2          # 1 col: exp scale 1/(Sq*Sk), replicated rows
O_SS = 3          # 1 col: sigmoid scale 1/(S1*S2)
O_B1 = 4          # 16 cols: S1*b1' replicated per batch  (m,b)
O_B2 = 20         # 64 cols: b2 replicated per batch      (kc,b)
O_MWC = 84        # 64 cols: mw*ca_gamma per (kc,b) [same for all kc]
O_MW = 148        # 64 cols: mw per (kc,b)
O_VBG = 212       # 64 cols: gamma*vb per (kc,b)
O_XM = 276        # 64 cols: spatial mean of xp per (kc,b)
NCONST = 340


def _up_matrix(n):
    """[2n, n] bilinear x2 upsample (align_corners=False, edge clamp)."""
    M = np.zeros((2 * n, n), np.float64)
    for o in range(2 * n):
        src = (o + 0.5) / 2.0 - 0.5
        i0 = int(np.floor(src))
        f = src - i0
        M[o, min(max(i0, 0), n - 1)] += 1.0 - f
        M[o, min(max(i0 + 1, 0), n - 1)] += f
    return M


def k_bilinear():
    """[384, 96] upsample matrix: flat(16,24) <- flat(8,12)."""
    return np.kron(_up_matrix(HD), _up_matrix(WD))


def split_excess_waits(nc, max_waits=1):
    """This walrus build rejects multi-sem-wait instructions on some opcodes;
    hoist extra waits onto preceding same-engine no-ops."""
    for f in nc.m.functions:
        for bb in f.blocks:
            insts = bb.instructions
            i = 0
            while i < len(insts):
                ins = insts[i]
                si = ins.sync_info
                if si is not None and si.on_wait and len(si.on_wait) > max_waits:
                    waits = list(si.on_wait)
                    extra, keep = waits[:-max_waits], waits[-max_waits:]
                    nops = []
                    for s in range(0, len(extra), max_waits):
                        nops.append(mybir.InstNoOp(
                            name=nc.get_next_instruction_name(),
                            engine=ins.engine, ins=[], outs=[],
                            sync_info=mybir.SyncInfo(
                                on_wait=extra[s:s + max_waits], on_update=[]),
                        ))
                    ins.sync_info = mybir.SyncInfo(
                        on_wait=keep, on_update=list(si.on_update or []))
                    insts[i:i] = nops
                    i += len(nops)
                i += 1


def build_program(split_waits=True):
    from contextlib import ExitStack
    nc = bass.Bass()

    x = nc.dram_tensor("x", [BL, C, H, W], BF16, kind="ExternalInput")
    xd = nc.dram_tensor("xd", [P, KC2, 128, 2, BL * N], FP8,
                        kind="ExternalInput")
    wqk = nc.dram_tensor("wqk", [P, KC2, 128, 2, 2 * IC], FP8,
                         kind="ExternalInput")
    wv = nc.dram_tensor("wv", [P, KC2, 128, 2, C], FP8, kind="ExternalInput")
    ktd = nc.dram_tensor("ktd", [P, N, HWP + 1], BF16, kind="ExternalInput")
    fc1 = nc.dram_tensor("fc1", [P, C, C4], FP8, kind="ExternalInput")
    fc2 = nc.dram_tensor("fc2", [P, C4, C], FP8, kind="ExternalInput")
    cst = nc.dram_tensor("cst", [P, 128, NCONST], F32, kind="ExternalInput")
    out = nc.dram_tensor("out", [BL, C, H, W], BF16, kind="ExternalOutput")

    xv = x.ap().rearrange("b (kc q) (p h) w -> b p q kc h w", kc=KC, p=P)
    ov = out.ap().rearrange("b (kc q) (p h) w -> b p q kc h w", kc=KC, p=P)
    wvv = wv.ap().rearrange("p (t k) q two d -> p t q k two d", t=4, k=2)
    wqkv = wqk.ap().rearrange("p kp q two i -> p q kp two i")
    fc1v = fc1.ap().rearrange("p (kc q) d -> p q kc d", kc=KC)
    fc2v = fc2.ap().rearrange("p (m q) c -> p q m c", m=4)

    with ExitStack() as ctx:
        tc = ctx.enter_context(tile.TileContext(nc))
        pool = lambda name, bufs, **kw: ctx.enter_context(
            tc.tile_pool(name=name, bufs=bufs, **kw))
        wv_pool = pool("wv", 4)
        wqk_pool = pool("wqk", 2)
        fc1_pool = pool("fc1", 2)
        fc2_pool = pool("fc2", 2)
        kt_pool = pool("ktp", 2)
        cst_pool = pool("cst", 2)
        xp_pool = pool("xp", 4)
        out_pool = pool("outp", 2)
        xd_pool = pool("xd", 2)
        qk_pool = pool("qk", 3)
        attn_pool = pool("attn", 4)
        g_pool = pool("gg", 5)
        vt_pool = pool("vt", 17)
        r_pool = pool("rr", 4)
        gap_pool = pool("gap", 18)
        se_pool = pool("se", 12)
        sm_pool = pool("sm", 10)
        ps_qk = pool("ps_qk", 2, space="PSUM")
        ps_b = pool("ps_b", 3, space="PSUM")
        ps_mu = pool("ps_mu", 1, space="PSUM")
        ps_up = pool("ps_up", 2, space="PSUM")

        def load_weights(p):
            xd_t = xd_pool.tile([128, KC2, 2, BL * N], FP8, tag="xd")
            nc.sync.dma_start(
                xd_t[:], xd.ap()[p].rearrange("kp q two n -> q kp two n"))
            wqk_t = wqk_pool.tile([128, KC2, 2, 2 * IC], FP8, tag="wqk")
            nc.sync.dma_start(wqk_t[:], wqkv[p])
            kt_t = kt_pool.tile([N, HWP + 1], BF16, tag="kt")
            nc.sync.dma_start(kt_t[:], ktd.ap()[p])
            cst_t = cst_pool.tile([128, NCONST], F32, tag="cst")
            nc.sync.dma_start(cst_t[:], cst.ap()[p])
            wv_t = []
            for t in range(4):
                w = wv_pool.tile([128, 2, 2, C], FP8, tag="wv")
                nc.sync.dma_start(w[:], wvv[p, t])
                wv_t.append(w)
            fc1_t = fc1_pool.tile([128, KC, C4], FP8, tag="fc1")
            nc.sync.dma_start(fc1_t[:], fc1v[p])
            fc2_t = fc2_pool.tile([128, 4, C], FP8, tag="fc2")
            nc.sync.dma_start(fc2_t[:], fc2v[p])
            return dict(xd=xd_t, wv=wv_t, wqk=wqk_t, fc1=fc1_t, fc2=fc2_t,
                        kt=kt_t, cst=cst_t)

        def load_xp(p):
            xp_t = []
            for b in range(BL):
                t = xp_pool.tile([128, KC, PH, W], BF16, tag="xp")
                nc.sync.dma_start(t[:], xv[b, p])
                xp_t.append(t)
            return xp_t

        wts = load_weights(0)
        xps = load_xp(0)
        for p in range(P):
            wv_t, wqk_t, fc1_t = wts["wv"], wts["wqk"], wts["fc1"]
            fc2_t, kt_t, cst_t = wts["fc2"], wts["kt"], wts["cst"]
            xd_b = wts["xd"][:]
            xp_t = xps

            # ---------- q/k projections (DoubleRow fp8, all BL) ----------
            q_ps = ps_qk.tile([IC, BL * N], F32, tag="qk")
            for kp in range(KC2):
                nc.tensor.matmul(q_ps[:], wqk_t[:, kp, :, 0:IC],
                                 xd_b[:, kp], start=(kp == 0),
                                 stop=(kp == KC2 - 1), perf_mode=DR)
            q_sb = qk_pool.tile([IC, BL * N], BF16, tag="qksb")
            nc.scalar.activation(q_sb[:], q_ps[:], AF.Identity,
                                 bias=cst_t[:, O_QB:O_QB + 1])
            k_ps = ps_qk.tile([IC, BL * N], F32, tag="qk")
            for kp in range(KC2):
                nc.tensor.matmul(k_ps[:], wqk_t[:, kp, :, IC:2 * IC],
                                 xd_b[:, kp], start=(kp == 0),
                                 stop=(kp == KC2 - 1), perf_mode=DR)
            k_sb = qk_pool.tile([IC, BL * N], BF16, tag="qksb")
            nc.scalar.activation(k_sb[:], k_ps[:], AF.Identity,
                                 bias=cst_t[:, O_KB:O_KB + 1])

            # ---------- per-batch: attention, G, vT, mean col ----------
            g_sb, vt_sb = [], []
            mu_ps = ps_mu.tile([128, KC * BL], F32, tag="mu")
            for b in range(BL):
                qs = q_sb[:, b * N:(b + 1) * N]
                ks = k_sb[:, b * N:(b + 1) * N]
                # softmax without max-shift: energy is descaled by exp's
                # input scale; |energy| ~ 1e-3 so exp cannot overflow
                e_ps = ps_b.tile([N, N], F32, tag="psb")
                nc.tensor.matmul(e_ps[:], qs, ks, start=True, stop=True)
                attn_e = attn_pool.tile([N, N], BF16, tag="attn")
                s_sum = sm_pool.tile([N, 1], F32, tag="sm")
                nc.scalar.activation(attn_e[:], e_ps[:], AF.Exp,
                                     scale=cst_t[0:N, O_ES:O_ES + 1],
                                     accum_out=s_sum[:])
                r_sum = sm_pool.tile([N, 1], F32, tag="sm")
                nc.vector.reciprocal(r_sum[:], s_sum[:])
                attn_n = attn_pool.tile([N, N], BF16, tag="attn")
                nc.vector.tensor_scalar(attn_n[:], attn_e[:], r_sum[:],
                                        None, ALU.mult)
                # G = attn_n @ KT  [N, 385]; col 384 = mean weights
                g_ps = ps_b.tile([N, HWP + 1], F32, tag="psb")
                nc.tensor.matmul(g_ps[:], attn_n[:], kt_t[:],
                                 start=True, stop=True)
                gsb = g_pool.tile([N, HWP + 1], BF16, tag="g")
                nc.scalar.activation(gsb[:], g_ps[:], AF.Copy)
                g_sb.append(gsb)
                # vT = xd_b^T @ WvT  [N, C]: DoubleRow fp8, chunk-outer,
                # one [N, 512] tile per chunk for fine-grained deps
                vchunks = []
                for bk in range(4):
                    vt_ps = ps_b.tile([N, 512], F32, tag="psb")
                    for kp in range(KC2):
                        nc.tensor.matmul(
                            vt_ps[:], xd_b[:, kp, :, b * N:(b + 1) * N],
                            wv_t[kp // 2][:, kp % 2, :,
                                          bk * 512:(bk + 1) * 512],
                            start=(kp == 0), stop=(kp == KC2 - 1),
                            perf_mode=DR)
                    vc = vt_pool.tile([N, 512], BF16, tag="vt")
                    if bk < 2:
                        nc.scalar.activation(vc[:], vt_ps[:], AF.Copy)
                    else:
                        nc.vector.tensor_scalar(vc[:], vt_ps[:], 1.0, None,
                                                ALU.mult)
                    vchunks.append(vc)
                vt_sb.append(vchunks)
                # mean column -> mu[:, kc*BL + b]
                for kc in range(KC):
                    nc.tensor.matmul(
                        mu_ps[:, kc * BL + b:kc * BL + b + 1],
                        vchunks[kc // 4][:, (kc % 4) * 128:(kc % 4 + 1) * 128],
                        gsb[:, HWP:HWP + 1], start=True, stop=True)

            # ---------- prefetch next part (weights + xp) ----------
            if p + 1 < P:
                wts = load_weights(p + 1)
                xps = load_xp(p + 1)

            # ---------- SE gate (all BL at once) ----------
            gap_t = []
            for kc in range(KC):
                g4 = gap_pool.tile([128, BL], BF16, tag="gap")
                nc.vector.tensor_tensor(
                    g4[:], mu_ps[:, kc * BL:(kc + 1) * BL],
                    cst_t[:, O_XM + kc * BL:O_XM + (kc + 1) * BL], ALU.add)
                gap_t.append(g4)
            h_ps = ps_b.tile([128, 4 * BL], F32, tag="psb")
            for m in range(4):
                for kc in range(KC):
                    nc.tensor.matmul(
                        h_ps[:, m * BL:(m + 1) * BL],
                        fc1_t[:, kc, m * 128:(m + 1) * 128], gap_t[kc][:],
                        start=(kc == 0), stop=(kc == KC - 1))
            h_pre = se_pool.tile([128, 4 * BL], F32, tag="se")
            nc.vector.tensor_tensor(h_pre[:], h_ps[:],
                                    cst_t[:, O_B1:O_B1 + 16], ALU.add)
            h1_t = se_pool.tile([128, 4 * BL], BF16, tag="se")
            nc.scalar.activation(h1_t[:], h_pre[:], AF.Relu)
            c_ps = ps_b.tile([128, KC * BL], F32, tag="psb")
            for kc in range(KC):
                for m in range(4):
                    nc.tensor.matmul(
                        c_ps[:, kc * BL:(kc + 1) * BL],
                        fc2_t[:, m, kc * 128:(kc + 1) * 128],
                        h1_t[:, m * BL:(m + 1) * BL],
                        start=(m == 0), stop=(m == 3))
            c_pre = se_pool.tile([128, KC * BL], F32, tag="se")
            nc.vector.scalar_tensor_tensor(
                c_pre[:], c_ps[:], cst_t[:, O_SS:O_SS + 1],
                cst_t[:, O_B2:O_B2 + 64], ALU.mult, ALU.add)
            cw_t = se_pool.tile([128, KC * BL], F32, tag="se")
            nc.scalar.activation(cw_t[:], c_pre[:], AF.Sigmoid)
            tmp_t = se_pool.tile([128, KC * BL], F32, tag="se")
            nc.vector.tensor_tensor(tmp_t[:], cw_t[:],
                                    cst_t[:, O_MWC:O_MWC + 64], ALU.mult)
            cw1_t = se_pool.tile([128, KC * BL], F32, tag="se")
            nc.vector.tensor_scalar(cw1_t[:], tmp_t[:], 1.0, None, ALU.add)
            cw2_t = se_pool.tile([128, KC * BL], F32, tag="se")
            nc.vector.tensor_tensor(cw2_t[:], tmp_t[:],
                                    cst_t[:, O_MW:O_MW + 64], ALU.add)
            bias2_t = se_pool.tile([128, KC * BL], F32, tag="se")
            nc.vector.tensor_tensor(bias2_t[:], cw2_t[:],
                                    cst_t[:, O_VBG:O_VBG + 64], ALU.mult)

            # ---------- up = vT^T @ G fused with blend + store ----------
            for b in range(BL):
                ot = out_pool.tile([128, KC, PH, W], BF16, tag="outp")
                for kc in range(KC):
                    col = kc * BL + b
                    up_ps = ps_up.tile([128, HWP], F32, tag="up")
                    nc.tensor.matmul(
                        up_ps[:],
                        vt_sb[b][kc // 4][:, (kc % 4) * 128:(kc % 4 + 1) * 128],
                        g_sb[b][:, 0:HWP], start=True, stop=True)
                    r = r_pool.tile([128, HWP], BF16, tag="r")
                    if kc % 2 == 0:
                        nc.scalar.activation(r[:], up_ps[:], AF.Identity,
                                             scale=cw2_t[:, col:col + 1],
                                             bias=bias2_t[:, col:col + 1])
                    else:
                        nc.vector.tensor_scalar(
                            r[:], up_ps[:], cw2_t[:, col:col + 1],
                            bias2_t[:, col:col + 1], ALU.mult, ALU.add)
                    nc.vector.scalar_tensor_tensor(
                        ot[:, kc].rearrange("q h w -> q (h w)"),
                        xp_t[b][:, kc].rearrange("q h w -> q (h w)"),
                        cw1_t[:, col:col + 1], r[:], ALU.mult, ALU.add)
                nc.sync.dma_start(ov[b, p], ot[:])

    if split_waits:
        split_excess_waits(nc)
    return nc


# ---------------------------------------------------------------------------
# Host side
# ---------------------------------------------------------------------------

def _sigmoid(v):
    return 1.0 / (1.0 + np.exp(-v))


def _bf(a):
    return np.ascontiguousarray(np.asarray(a).astype(ml_dtypes.bfloat16))


def _f32(a):
    return np.ascontiguousarray(np.asarray(a, dtype=np.float32))


def _q8(w):
    """Quantize to fp8 e4m3 with a power-of-2 scale; returns (w8, scale)."""
    w = np.asarray(w, dtype=np.float64)
    amax = np.abs(w).max()
    if amax == 0.0:
        return w.astype(ml_dtypes.float8_e4m3), 1.0
    s = 2.0 ** np.floor(np.log2(224.0 / amax))
    w8 = np.clip(w * s, -224.0, 224.0).astype(ml_dtypes.float8_e4m3)
    return w8, s


def prepare_host_inputs(inputs):
    """Fold/transpose/quantize weights; returns per-core input dicts."""
    g = {k: np.asarray(v) for k, v in inputs.items()}
    x = np.asarray(g["x"])

    # modality gate on host (tiny): mw [B, P]
    mf = g["modality"].astype(np.float64)[:, None]
    g1 = np.maximum(mf @ g["gate_w1"].astype(np.float64).T
                    + g["gate_b1"].astype(np.float64), 0.0)
    mw = _sigmoid(g1 @ g["gate_w2"].astype(np.float64).T
                  + g["gate_b2"].astype(np.float64))      # [B, P]

    paq = g["pa_q_w"].astype(np.float64)    # [P, IC, C]
    pak = g["pa_k_w"].astype(np.float64)
    pav = g["pa_v_w"].astype(np.float64)    # [P, C, C]
    dwq_w = g["pa_dw_q_w"].astype(np.float64)   # [P, C]
    dwq_b = g["pa_dw_q_b"].astype(np.float64)
    dwk_w = g["pa_dw_k_w"].astype(np.float64)
    dwk_b = g["pa_dw_k_b"].astype(np.float64)
    gam = g["pa_gamma"].astype(np.float64)      # [P]
    cgam = g["ca_gamma"].astype(np.float64)
    fc1w = g["ca_fc1_w"].astype(np.float64)     # [P, C4, C]
    fc2w = g["ca_fc2_w"].astype(np.float64)     # [P, C, C4]
    kb_mat = k_bilinear()                       # [384, 96]

    wqk8 = np.empty((P, C, 2 * IC), ml_dtypes.float8_e4m3)
    wv8 = np.empty((P, C, C), ml_dtypes.float8_e4m3)
    sxd_pre = np.asarray(inputs["x"], dtype=np.float32)
    xds_pre = _bf(sxd_pre).astype(np.float32).reshape(
        B, C, P, HD, 2, WD, 2).sum(axis=(4, 6))
    sxd = np.array([
        2.0 ** np.floor(np.log2(224.0 / max(np.abs(xds_pre[:, :, p]).max(),
                                            1e-30)))
        for p in range(P)])
    fc1_8 = np.empty((P, C, C4), ml_dtypes.float8_e4m3)
    fc2_8 = np.empty((P, C4, C), ml_dtypes.float8_e4m3)
    ktd = np.empty((P, N, HWP + 1), np.float64)
    cst = np.zeros((P, 128, NCONST), np.float32)

    for p in range(P):
        wqT = (paq[p] * dwq_w[p][None, :] * 0.25).T      # [C, IC]
        wkT = (pak[p] * dwk_w[p][None, :] * 0.25).T
        qb = g["pa_q_b"][p] + paq[p] @ dwq_b[p]          # [IC]
        kb = g["pa_k_b"][p] + pak[p] @ dwk_b[p]
        wq8, sq = _q8(wqT)
        wk8, sk = _q8(wkT)
        wqk8[p, :, 0:IC] = wq8
        wqk8[p, :, IC:2 * IC] = wk8
        cst[p, :, O_QB] = sq * sxd[p] * qb
        cst[p, :, O_KB] = sk * sxd[p] * kb
        cst[p, :, O_ES] = 1.0 / (sq * sk * sxd[p] * sxd[p])

        wvT = 0.25 * pav[p].T                            # [C, C]
        wv8[p], sv = _q8(wvT)
        # KT (gamma folded, descale by sv) + mean column
        ktd[p] = (gam[p] / (sv * sxd[p])) * np.concatenate(
            [kb_mat.T, kb_mat.mean(axis=0)[:, None]], axis=1)

        vbg = gam[p] * g["pa_v_b"][p].astype(np.float64)  # [C]
        b1 = g["ca_fc1_b"][p] + fc1w[p] @ vbg             # [C4]
        fc1_8[p], s1 = _q8(fc1w[p].T)
        fc2_8[p], s2 = _q8(fc2w[p].T)
        cst[p, :, O_SS] = 1.0 / (s1 * s2)
        for m in range(4):
            for b in range(BL):
                cst[p, :, O_B1 + m * BL + b] = s1 * b1[m * 128:(m + 1) * 128]
        b2 = g["ca_fc2_b"][p].astype(np.float64)
        for kc in range(KC):
            for b in range(BL):
                cst[p, :, O_B2 + kc * BL + b] = b2[kc * 128:(kc + 1) * 128]
                cst[p, :, O_VBG + kc * BL + b] = vbg[kc * 128:(kc + 1) * 128]

    def _pairs(w8, last):
        # [P, C, last] -> [P, KC2, 128, 2, last] pairing channel blocks
        return np.ascontiguousarray(
            w8.reshape(P, KC2, 2, 128, last).transpose(0, 1, 3, 2, 4))

    shared = {
        "wqk": _pairs(wqk8, 2 * IC),
        "wv": _pairs(wv8, C),
        "fc1": np.ascontiguousarray(fc1_8),
        "fc2": np.ascontiguousarray(fc2_8),
        "ktd": _bf(ktd),
    }
    # host 2x2 sum-pool (0.25 folded into weights) + per-part spatial mean
    xb = _bf(x)                                           # [B, C, H, W] bf16
    xr = xb.astype(np.float32).reshape(B, C, P, HD, 2, WD, 2)
    xds = xr.sum(axis=(4, 6))                             # [B, C, P, HD, WD]

    xmean = x.astype(np.float64).reshape(
        B, C, P, PH * W).mean(axis=3).astype(np.float32)  # [B, C, P]
    per_core = []
    for cid in range(N_CORES):
        mwl = mw[cid * BL:(cid + 1) * BL]    # [BL, P]
        cstc = cst.copy()
        xdl = xds[cid * BL:(cid + 1) * BL]   # [BL, C, P, HD, WD]
        # xd layout [P, KC2, 128, 2, BL*N], fp8 with per-part scale
        xdq = (xdl.reshape(BL, KC, 128, P, N).transpose(3, 1, 2, 0, 4)
               .reshape(P, KC, 128, BL * N) * sxd[:, None, None, None])
        xdc = np.ascontiguousarray(
            xdq.reshape(P, KC2, 2, 128, BL * N).transpose(0, 1, 3, 2, 4)
            .astype(ml_dtypes.float8_e4m3))
        xml = xmean[cid * BL:(cid + 1) * BL]  # [BL, C, P]
        for p in range(P):
            for kc in range(KC):
                for b in range(BL):
                    cstc[p, :, O_MWC + kc * BL + b] = mwl[b, p] * cgam[p]
                    cstc[p, :, O_MW + kc * BL + b] = mwl[b, p]
                    cstc[p, :, O_XM + kc * BL + b] = \
                        xml[b, kc * 128:(kc + 1) * 128, p]
        per_core.append({
            "x": xb[cid * BL:(cid + 1) * BL],
            "xd": xdc,
            "cst": cstc,
            **shared,
        })
    return per_core


_CACHE = {}


def kernel(**inputs):
    from concourse.bass_utils import run_bass_kernel_spmd

    per_core = prepare_host_inputs(inputs)
    if "nc" not in _CACHE:
        _CACHE["nc"] = build_program()
    nc = _CACHE["nc"]
    res = run_bass_kernel_spmd(nc, per_core, list(range(N_CORES)))
    outs = [res.results[c]["out"] for c in range(N_CORES)]
    return np.concatenate(outs, axis=0).astype(np.float32)
